# revision 2
# baseline (speedup 1.0000x reference)
"""Trainium2 Bass kernel for nn_MultiHeadAttentionQuantum (v3).

Math: the per-(batch,token,head) quantum circuit (RX(x_i+theta_i) encode, CNOT
ring, <Z_i> readout) collapses analytically to cosine prefix-products:
    <Z_0> = prod_{i=1..7} cos(x_i + theta_i)
    <Z_w> = prod_{i=0..w} cos(x_i + theta_i)   (w >= 1)
so the quantum head xq is host-precomputable input prep (like the baseline's
host-encoded cos angles). The device runs the S^2 work: 16-head self-attention
(q=k=v=xq, d_k=8, no max-subtraction: |score| <= sqrt(8)) + output projection.

Device structure per core (1 batch element, S=512 as 4 blocks x 128):
 - scores: fp8e4 DoubleRow matmuls (0.5 cyc/row). Only the upper-triangle
   block-tiles (c<=j, 10 of 16 per head) are computed; scores are symmetric.
 - exp: ONE [128,1280] activation per head, PSUM->SBUF fp16. The ACT engine is
   the roofline: 16 x (1280 x 0.83ns + 185ns).
 - lower-triangle tiles: batched DMA-engine transpose (XBAR) of the 6 exp'd
   off-diagonal tiles -> TB; no compute-engine time. The last head transposes
   on the (by then idle) PE instead, skipping the DMA latency.
 - PV in [q, d] orientation: per (head, q-block, k-block) one matmul with the
   P tile as stationary lhsT and the 9-wide V slab as moving rhs; accumulates
   into psUA[q, 256a+9h+d]. The ones column d=8 gives the softmax Z per
   (head, q) ON THE PARTITION AXIS, so normalization is one strided
   reciprocal + one stride-0-broadcast multiply per head - no Z-broadcast
   matmuls at all.
 - tail per q-block: transpose xo -> project -> ship. q-block 3 takes no
   transposed tiles and finishes early.

Sharding: data-parallel over batch, one batch element per NeuronCore (B=8).
"""

import math
import sys

sys.path.insert(0, "/opt/trn_rl_repo")

import numpy as np
import ml_dtypes

import concourse.bass as bass  # noqa: F401
import concourse.tile as tile
from concourse import bacc, mybir
from concourse import bass_utils

FP32 = mybir.dt.float32
FP16 = mybir.dt.float16
FP8 = mybir.dt.float8e4
AF = mybir.ActivationFunctionType
DR = mybir.MatmulPerfMode.DoubleRow

B, S, E, H, NW = 8, 512, 128, 16, 8   # batch, seq, embed, heads, wires(d_k)
TB = S // 128                         # token blocks = 4
ISQ = 1.0 / math.sqrt(NW)

# upper off-diagonal tile order (c, j), j > c — fixed layout everywhere
UPPERS = [(0, 1), (0, 2), (0, 3), (1, 2), (1, 3), (2, 3)]
UIDX = {t: i for i, t in enumerate(UPPERS)}

_CACHE = {}


def build(repeat: int = 1):
    if repeat in _CACHE:
        return _CACHE[repeat]

    nc = bacc.Bacc("TRN2", target_bir_lowering=False, debug=False, num_devices=8)

    q2t_d = nc.dram_tensor("q2t", [4, H * 1024], FP8, kind="ExternalInput").ap()
    vp_d = nc.dram_tensor("vp", [128, TB * H * 9], FP16, kind="ExternalInput").ap()
    # merged consts: [:,0:128] W^T | [:,128:256] identity | [0,256:384] ones |
    # [0,384:512] bias | [0,512:640] zeros
    cst_d = nc.dram_tensor("cst", [128, 640], FP16, kind="ExternalInput").ap()
    # output: [q-in-block, (qblock, e')]
    yout_d = nc.dram_tensor("yout", [128, 512], FP32, kind="ExternalOutput").ap()

    with tile.TileContext(nc) as tc:
        with tc.tile_pool(name="consts", bufs=1) as cpool, \
             tc.tile_pool(name="pd", bufs=4) as pdpool, \
             tc.tile_pool(name="tb", bufs=4) as tbpool, \
             tc.tile_pool(name="psS", bufs=2, space="PSUM") as psS, \
             tc.tile_pool(name="psUA", bufs=1, space="PSUM") as psUAp:

            for rep in range(repeat):
                Q2 = cpool.tile([4, H * 1024], FP8, tag="Q2")
                nc.sync.dma_start(Q2[:], q2t_d[:])
                cst = cpool.tile([128, 640], FP16, tag="cst")
                nc.sync.dma_start(cst[:], cst_d[:])
                VP = cpool.tile([128, TB * H * 9], FP16, tag="VP")
                nc.sync.dma_start(VP[:], vp_d[:])

                w2 = cst[:, 0:128]
                idn = cst[:, 128:256]
                ones_r = cst[0:1, 256:384]
                bvec_r = cst[0:1, 384:512]
                zero_r = cst[0:1, 512:640]

                # attention accumulator, [q, (a: 256-pad)(h: 9)]: d<8 = PV,
                # d=8 = Z. Lives for the whole kernel (2 banks).
                psUA = psUAp.tile([128, 1024], FP32, tag="psUA", name="psUA")
                # warm-up matmuls on a dependency-free zero tile: they keep the
                # PE p-state ramp alive until the first scores arrive, and
                # their start=True marks pending-zero over both psUA banks
                # (all PV matmuls then accumulate with start=False)
                wz = cpool.tile([128, 512], FP16, tag="wz")
                nc.gpsimd.memset(wz[:], 0.0)
                for col in (0, 512):
                    nc.tensor.matmul(psUA[:, col:col + 1], wz[0:1, 0:128],
                                     wz[0:1, 0:1], start=True, stop=True,
                                     skip_group_check=True)

                # normalized attention output [q, (a, h, d<8)] fp16
                xo = cpool.tile([128, 512], FP16, tag="xo")
                # per-(head, qblock) reciprocal softmax denominators
                rz = cpool.tile([128, 64], FP32, tag="rz")

                P16 = {}
                TBs = {}

                def q2h(h):
                    # [4, 2, 512] fp8 view of head h (pair dim = wire halves)
                    return Q2[0:4, 1024 * h:1024 * (h + 1)].rearrange(
                        "p (i n) -> p i n", i=2)

                def vpc(h, c):
                    return VP[:, 144 * c + 9 * h:144 * c + 9 * h + 9]

                def emit_scores(h):
                    ps = psS.tile([128, 1536], FP32, tag="psS", name=f"psS{h}",
                                  padded_shape=[128, 1536])
                    q2 = q2h(h)
                    lhsT = lambda c: q2[:, :, 128 * c:128 * (c + 1)]
                    # bank0 [0,512): diagonals; bank1 [512,1024): uppers u0-u3;
                    # bank2 [1024,1536): uppers u4,u5. First matmul emitted per
                    # bank carries start=True (pending-zero covers the bank).
                    mm = nc.tensor.matmul
                    mm(ps[:, 0:128], lhsT(0), q2[:, :, 0:128],
                       start=True, stop=True, perf_mode=DR, skip_group_check=True)
                    mm(ps[:, 512:896], lhsT(0), q2[:, :, 128:512],
                       start=True, stop=True, perf_mode=DR, skip_group_check=True)
                    mm(ps[:, 1024:1152], lhsT(1), q2[:, :, 384:512],
                       start=True, stop=True, perf_mode=DR, skip_group_check=True)
                    for c in (1, 2, 3):
                        mm(ps[:, 128 * c:128 * (c + 1)], lhsT(c),
                           q2[:, :, 128 * c:128 * (c + 1)],
                           start=False, stop=True, perf_mode=DR,
                           skip_group_check=True)
                    mm(ps[:, 896:1024], lhsT(1), q2[:, :, 256:384],
                       start=False, stop=True, perf_mode=DR, skip_group_check=True)
                    mm(ps[:, 1152:1280], lhsT(2), q2[:, :, 384:512],
                       start=False, stop=True, perf_mode=DR, skip_group_check=True)
                    return ps

                def emit_exp(h, ps):
                    p16 = pdpool.tile([128, 1280], FP16, tag="P16", name=f"P16_{h}")
                    if h == H - 1:
                        # split: uppers first so the tail transpose can start
                        # before the diagonals are exp'd
                        nc.scalar.activation(p16[:, 512:1280], ps[:, 512:1280],
                                             AF.Exp, scale=ISQ)
                        nc.scalar.activation(p16[:, 0:512], ps[:, 0:512],
                                             AF.Exp, scale=ISQ)
                    else:
                        nc.scalar.activation(p16[:], ps[:, 0:1280], AF.Exp,
                                             scale=ISQ)
                    P16[h] = p16

                def emit_transpose(h):
                    # lower-triangle tiles via batched XBAR DMA transpose
                    tb = tbpool.tile([128, 768], FP16, tag="TB", name=f"TB{h}")
                    nc.sync.dma_start(
                        tb[:].rearrange("p (i q) -> p i q", i=6),
                        P16[h][:, 512:1280], transpose=True)
                    TBs[h] = tb

                def emit_transpose_pe_mm(h, psT):
                    # PE-side transpose of the 6 upper tiles (for the last
                    # head: skips the XBAR DMA latency; PE is idle by then)
                    for u in range(6):
                        nc.tensor.transpose(psT[:, 128 * u:128 * (u + 1)],
                                            P16[h][:, 512 + 128 * u:640 + 128 * u],
                                            idn)

                def emit_transpose_pe_evac(h, psT):
                    tb = tbpool.tile([128, 768], FP16, tag="TB", name=f"TB{h}")
                    nc.vector.tensor_copy(tb[:], psT[:, 0:768])
                    TBs[h] = tb

                def ptile(h, c, a):
                    """P tile [k-in-block-c (part), q-in-block-a (free)]."""
                    p16 = P16[h]
                    if c == a:
                        return p16[:, 128 * c:128 * (c + 1)]
                    if c < a:
                        u = UIDX[(c, a)]
                        return p16[:, 512 + 128 * u:640 + 128 * u]
                    u = UIDX[(a, c)]
                    return TBs[h][:, 128 * u:128 * (u + 1)]

                def emit_pv(h, ablocks):
                    for a in ablocks:
                        base = 256 * a + 9 * h
                        for c in range(4):
                            nc.tensor.matmul(
                                psUA[:, base:base + 9], ptile(h, c, a),
                                vpc(h, c), start=False, stop=(c == 3),
                                skip_group_check=True)

                def emit_norm(h, ablocks):
                    a0, n = min(ablocks), len(ablocks)
                    zin = psUA[:].rearrange("p (a z) -> p a z", a=4)[
                        :, a0:a0 + n, 9 * h + 8]
                    nc.vector.reciprocal_approx_fast(
                        out=rz[:].rearrange("p (hh a) -> p hh a", hh=16)[
                            :, h, a0:a0 + n],
                        in_=zin)
                    o_in = psUA[:].rearrange("p (a z) -> p a z", a=4)[
                        :, a0:a0 + n, 9 * h:9 * h + 8]
                    rzb = rz[:].rearrange("p (hh a d) -> p hh a d", hh=16, a=4)[
                        :, h, a0:a0 + n, :].broadcast_to([128, n, 8])
                    xov = xo[:].rearrange("p (a e) -> p a e", a=4)[
                        :, a0:a0 + n, 8 * h:8 * h + 8]
                    nc.vector.tensor_tensor(xov, o_in, rzb, mybir.AluOpType.mult)

                def emit_norm3():
                    # merged normalization for heads 13..15 (the tail): one
                    # strided reciprocal + one broadcast multiply instead of
                    # three serialized per-head pairs
                    za = psUA[:].rearrange("p (a z) -> p a z", a=4)[
                        :, :, 117:144].rearrange("p a (hh d) -> p a hh d", hh=3)
                    rzv = rz[:, 52:64].rearrange("p (hh a) -> p a hh", hh=3)
                    nc.vector.reciprocal_approx_fast(
                        out=rzv, in_=za[:, :, :, 8])
                    rzb = rz[:, 52:64].rearrange(
                        "p (hh a d) -> p a hh d", hh=3, a=4).broadcast_to(
                        [128, 4, 3, 8])
                    xov = xo[:].rearrange("p (a e) -> p a e", a=4)[
                        :, :, 104:128].rearrange("p a (hh d) -> p a hh d", hh=3)
                    nc.vector.tensor_tensor(xov, za[:, :, :, 0:8], rzb,
                                            mybir.AluOpType.mult)

                def emit_out(ablocks, psTx, psOA, tag):
                    # transpose xo block -> project -> ship, per q-block
                    for a in ablocks:
                        nc.tensor.transpose(psTx[:, 128 * a:128 * (a + 1)],
                                            xo[:, 128 * a:128 * (a + 1)], idn)
                    xoT = cpool.tile([128, 512], FP16, tag=f"xoT{tag}")
                    lo, hi = 128 * min(ablocks), 128 * (max(ablocks) + 1)
                    nc.vector.tensor_copy(xoT[:, lo:hi], psTx[:, lo:hi])
                    for a in ablocks:
                        nc.tensor.matmul(psOA[:, 128 * a:128 * (a + 1)],
                                         xoT[:, 128 * a:128 * (a + 1)], w2,
                                         start=True, stop=False,
                                         skip_group_check=True)
                        nc.tensor.matmul(psOA[:, 128 * a:128 * (a + 1)],
                                         ones_r, bvec_r,
                                         start=False, stop=True,
                                         skip_group_check=True)
                    yo = cpool.tile([128, 512], FP32, tag=f"yo{tag}")
                    nc.vector.tensor_copy(yo[:, lo:hi], psOA[:, lo:hi])
                    nc.sync.dma_start(yout_d[:, lo:hi], yo[:, lo:hi])

                # ---- pipelined head loop
                LAG = 3
                for h in range(H + LAG + 1):
                    if h < H:
                        ps = emit_scores(h)
                        emit_exp(h, ps)
                        if h < H - 2:
                            emit_transpose(h)
                    hp = h - LAG
                    if 0 <= hp < H - 2:
                        emit_pv(hp, (0, 1, 2, 3))
                        if hp < H - 3:
                            emit_norm(hp, (0, 1, 2, 3))
                    elif hp == H - 2:
                        # head 14: PE-side transpose too (its XBAR DMA would
                        # land past the end of the exp stream). Ring slot
                        # waits psS14's release = exp(14) - exactly its gate.
                        psT14 = psS.tile([128, 768], FP16, tag="psS",
                                         name="psT14")
                        emit_transpose_pe_mm(hp, psT14)
                        emit_transpose_pe_evac(hp, psT14)
                        emit_pv(hp, (0, 1, 2, 3))
                    elif hp == H - 1:
                        # tail: PE-side transpose for the last head, then one
                        # combined normalize/project/ship chain. Ring slots
                        # (bufs=2): psT15 <- psS15 (exp15), psTx <- psT14
                        # (its evac), psOA <- psT15 (its evac) - no cycles.
                        psT15 = psS.tile([128, 768], FP16, tag="psS",
                                         name="psT15")
                        psTx = psS.tile([128, 512], FP16, tag="psS",
                                        name="psTx")
                        psOA = psS.tile([128, 512], FP32, tag="psS",
                                        name="psOA")
                        emit_transpose_pe_mm(hp, psT15)
                        emit_transpose_pe_evac(hp, psT15)
                        emit_pv(hp, (0, 1, 2, 3))
                        emit_norm3()
                        emit_out((0, 1, 2, 3), psTx, psOA, "t")

    nc.compile()
    _CACHE[repeat] = nc
    return nc


def _xq(x: np.ndarray, theta: np.ndarray) -> np.ndarray:
    """Analytic quantum-head output: cosine prefix products, [B, S, H, NW]."""
    th = np.tile(theta.astype(np.float64), E // NW)
    C = np.cos(x.astype(np.float64) + th).reshape(B, S, H, NW)
    XQ = np.cumprod(C, axis=-1)
    XQ[..., 0] = np.prod(C[..., 1:], axis=-1)
    return XQ.astype(np.float32)


def _consts(W: np.ndarray, b: np.ndarray):
    cst = np.zeros((128, 640), dtype=np.float16)
    cst[:, 0:128] = W.astype(np.float16).T          # W2[8h+d, e'] = W[e', 8h+d]
    cst[:, 128:256] = np.eye(128, dtype=np.float16)  # identity for transposes
    cst[0, 256:384] = 1.0                            # ones row
    cst[0, 384:512] = b.astype(np.float16)           # bias
    return {"cst": cst}


def _prep_x(x: np.ndarray, theta: np.ndarray) -> list[dict]:
    """Per-core tensors: Q2T fp8 score operands + VP fp16 value slabs."""
    xq = _xq(x, theta)  # [B, S, H, NW]
    out = []
    for bb in range(B):
        xqb = xq[bb]  # [S, H, NW]
        # q2t[p, 1024h + 512i + s] = xq[s, h, 4i+p]
        q2 = xqb.astype(ml_dtypes.float8_e4m3)  # [S, H, NW]
        q2t = q2.transpose(1, 2, 0).reshape(H, 2, 4, S)  # [h, i, p, s]
        q2t = np.ascontiguousarray(q2t.transpose(2, 0, 1, 3)).reshape(4, H * 1024)
        # vp[p, 144c + 9h + d] = xq[128c+p, h, d] (d<8); 1.0 at d=8
        vp = np.ones((TB, 128, H, 9), dtype=np.float16)
        vp[:, :, :, 0:8] = xqb.reshape(TB, 128, H, NW).astype(np.float16)
        vp = np.ascontiguousarray(vp.transpose(1, 0, 2, 3)).reshape(128, TB * H * 9)
        out.append({"q2t": q2t, "vp": vp})
    return out


def _in_maps(x, theta, W, b):
    consts = _consts(W, b)
    per_core = _prep_x(x, theta)
    return [{**consts, **pc} for pc in per_core]


def kernel(x: np.ndarray, theta: np.ndarray, W: np.ndarray, b: np.ndarray) -> np.ndarray:
    x = np.asarray(x, dtype=np.float32)
    theta = np.asarray(theta, dtype=np.float32)
    W = np.asarray(W, dtype=np.float32)
    b = np.asarray(b, dtype=np.float32)

    nc = build(repeat=1)
    in_maps = _in_maps(x, theta, W, b)
    res = bass_utils.run_bass_kernel_spmd(nc, in_maps, core_ids=list(range(8)))

    y = np.empty((B, S, E), dtype=np.float32)
    for c in range(B):
        # yout[p, 128a + e'] -> y[128a + p, e']
        y[c] = res.results[c]["yout"].reshape(128, 4, 128).transpose(
            1, 0, 2).reshape(S, E)
    return y


# revision 3
# speedup vs baseline: 1.0113x; 1.0113x over previous
"""Trainium2 Bass kernel for nn_MultiHeadAttentionQuantum (v3).

Math: the per-(batch,token,head) quantum circuit (RX(x_i+theta_i) encode, CNOT
ring, <Z_i> readout) collapses analytically to cosine prefix-products:
    <Z_0> = prod_{i=1..7} cos(x_i + theta_i)
    <Z_w> = prod_{i=0..w} cos(x_i + theta_i)   (w >= 1)
so the quantum head xq is host-precomputable input prep (like the baseline's
host-encoded cos angles). The device runs the S^2 work: 16-head self-attention
(q=k=v=xq, d_k=8, no max-subtraction: |score| <= sqrt(8)) + output projection.

Device structure per core (1 batch element, S=512 as 4 blocks x 128):
 - scores: fp8e4 DoubleRow matmuls (0.5 cyc/row). Only the upper-triangle
   block-tiles (c<=j, 10 of 16 per head) are computed; scores are symmetric.
 - exp: ONE [128,1280] activation per head, PSUM->SBUF fp16. The ACT engine is
   the roofline: 16 x (1280 x 0.83ns + 185ns).
 - lower-triangle tiles: batched DMA-engine transpose (XBAR) of the 6 exp'd
   off-diagonal tiles -> TB; no compute-engine time. The last head transposes
   on the (by then idle) PE instead, skipping the DMA latency.
 - PV in [q, d] orientation: per (head, q-block, k-block) one matmul with the
   P tile as stationary lhsT and the 9-wide V slab as moving rhs; accumulates
   into psUA[q, 256a+9h+d]. The ones column d=8 gives the softmax Z per
   (head, q) ON THE PARTITION AXIS, so normalization is one strided
   reciprocal + one stride-0-broadcast multiply per head - no Z-broadcast
   matmuls at all.
 - tail per q-block: transpose xo -> project -> ship. q-block 3 takes no
   transposed tiles and finishes early.

Sharding: data-parallel over batch, one batch element per NeuronCore (B=8).
"""

import math
import sys

sys.path.insert(0, "/opt/trn_rl_repo")

import numpy as np
import ml_dtypes

import concourse.bass as bass  # noqa: F401
import concourse.tile as tile
from concourse import bacc, mybir
from concourse import bass_utils

FP32 = mybir.dt.float32
FP16 = mybir.dt.float16
FP8 = mybir.dt.float8e4
AF = mybir.ActivationFunctionType
DR = mybir.MatmulPerfMode.DoubleRow

B, S, E, H, NW = 8, 512, 128, 16, 8   # batch, seq, embed, heads, wires(d_k)
TB = S // 128                         # token blocks = 4
ISQ = 1.0 / math.sqrt(NW)

# upper off-diagonal tile order (c, j), j > c — fixed layout everywhere
UPPERS = [(0, 1), (0, 2), (0, 3), (1, 2), (1, 3), (2, 3)]
UIDX = {t: i for i, t in enumerate(UPPERS)}

_CACHE = {}


def build(repeat: int = 1):
    if repeat in _CACHE:
        return _CACHE[repeat]

    nc = bacc.Bacc("TRN2", target_bir_lowering=False, debug=False, num_devices=8)

    q2t_d = nc.dram_tensor("q2t", [4, H * 1024], FP8, kind="ExternalInput").ap()
    vp_d = nc.dram_tensor("vp", [128, TB * H * 9], FP16, kind="ExternalInput").ap()
    # merged consts: [:,0:128] W^T | [:,128:256] identity | [0,256:384] ones |
    # [0,384:512] bias | [0,512:640] zeros
    cst_d = nc.dram_tensor("cst", [128, 640], FP16, kind="ExternalInput").ap()
    # output: [q-in-block, (qblock, e')]
    yout_d = nc.dram_tensor("yout", [128, 512], FP32, kind="ExternalOutput").ap()

    with tile.TileContext(nc) as tc:
        with tc.tile_pool(name="consts", bufs=1) as cpool, \
             tc.tile_pool(name="pd", bufs=4) as pdpool, \
             tc.tile_pool(name="tb", bufs=4) as tbpool, \
             tc.tile_pool(name="psS", bufs=2, space="PSUM") as psS, \
             tc.tile_pool(name="psUA", bufs=1, space="PSUM") as psUAp:

            for rep in range(repeat):
                Q2 = cpool.tile([4, H * 1024], FP8, tag="Q2")
                nc.sync.dma_start(Q2[:], q2t_d[:])
                cst = cpool.tile([128, 640], FP16, tag="cst")
                nc.sync.dma_start(cst[:], cst_d[:])
                VP = cpool.tile([128, TB * H * 9], FP16, tag="VP")
                nc.sync.dma_start(VP[:], vp_d[:])

                w2 = cst[:, 0:128]
                idn = cst[:, 128:256]
                ones_r = cst[0:1, 256:384]
                bvec_r = cst[0:1, 384:512]
                zero_r = cst[0:1, 512:640]

                # attention accumulator, [q, (a: 256-pad)(h: 9)]: d<8 = PV,
                # d=8 = Z. Lives for the whole kernel (2 banks).
                psUA = psUAp.tile([128, 1024], FP32, tag="psUA", name="psUA")
                # warm-up matmuls on a dependency-free zero tile: they keep the
                # PE p-state ramp alive until the first scores arrive, and
                # their start=True marks pending-zero over both psUA banks
                # (all PV matmuls then accumulate with start=False)
                wz = cpool.tile([128, 512], FP16, tag="wz")
                nc.gpsimd.memset(wz[:], 0.0)
                for col in (0, 512):
                    nc.tensor.matmul(psUA[:, col:col + 1], wz[0:1, 0:128],
                                     wz[0:1, 0:1], start=True, stop=True,
                                     skip_group_check=True)

                # normalized attention output [q, (a, h, d<8)] fp16
                xo = cpool.tile([128, 512], FP16, tag="xo")
                # per-(head, qblock) reciprocal softmax denominators
                rz = cpool.tile([128, 64], FP32, tag="rz")

                P16 = {}
                TBs = {}
                psT2 = [None]

                def q2h(h):
                    # [4, 2, 512] fp8 view of head h (pair dim = wire halves)
                    return Q2[0:4, 1024 * h:1024 * (h + 1)].rearrange(
                        "p (i n) -> p i n", i=2)

                def vpc(h, c):
                    return VP[:, 144 * c + 9 * h:144 * c + 9 * h + 9]

                def emit_scores(h):
                    ps = psS.tile([128, 1536], FP32, tag="psS", name=f"psS{h}",
                                  padded_shape=[128, 1536])
                    q2 = q2h(h)
                    lhsT = lambda c: q2[:, :, 128 * c:128 * (c + 1)]
                    # bank0 [0,512): diagonals; bank1 [512,1024): uppers u0-u3;
                    # bank2 [1024,1536): uppers u4,u5. First matmul emitted per
                    # bank carries start=True (pending-zero covers the bank).
                    mm = nc.tensor.matmul
                    mm(ps[:, 0:128], lhsT(0), q2[:, :, 0:128],
                       start=True, stop=True, perf_mode=DR, skip_group_check=True)
                    mm(ps[:, 512:896], lhsT(0), q2[:, :, 128:512],
                       start=True, stop=True, perf_mode=DR, skip_group_check=True)
                    mm(ps[:, 1024:1152], lhsT(1), q2[:, :, 384:512],
                       start=True, stop=True, perf_mode=DR, skip_group_check=True)
                    for c in (1, 2, 3):
                        mm(ps[:, 128 * c:128 * (c + 1)], lhsT(c),
                           q2[:, :, 128 * c:128 * (c + 1)],
                           start=False, stop=True, perf_mode=DR,
                           skip_group_check=True)
                    mm(ps[:, 896:1024], lhsT(1), q2[:, :, 256:384],
                       start=False, stop=True, perf_mode=DR, skip_group_check=True)
                    mm(ps[:, 1152:1280], lhsT(2), q2[:, :, 384:512],
                       start=False, stop=True, perf_mode=DR, skip_group_check=True)
                    return ps

                def emit_exp(h, ps):
                    p16 = pdpool.tile([128, 1280], FP16, tag="P16", name=f"P16_{h}")
                    nc.scalar.activation(p16[:], ps[:, 0:1280], AF.Exp,
                                         scale=ISQ)
                    P16[h] = p16

                def emit_transpose(h):
                    # lower-triangle tiles via batched XBAR DMA transpose
                    tb = tbpool.tile([128, 768], FP16, tag="TB", name=f"TB{h}")
                    nc.sync.dma_start(
                        tb[:].rearrange("p (i q) -> p i q", i=6),
                        P16[h][:, 512:1280], transpose=True)
                    TBs[h] = tb

                def emit_transpose_pe_mm(h, psT):
                    # PE-side transpose of the 6 upper tiles (for the last
                    # head: skips the XBAR DMA latency; PE is idle by then)
                    for u in range(6):
                        nc.tensor.transpose(psT[:, 128 * u:128 * (u + 1)],
                                            P16[h][:, 512 + 128 * u:640 + 128 * u],
                                            idn)

                def emit_transpose_pe_evac(h, psT):
                    tb = tbpool.tile([128, 768], FP16, tag="TB", name=f"TB{h}")
                    nc.vector.tensor_copy(tb[:], psT[:, 0:768])
                    TBs[h] = tb

                def ptile(h, c, a):
                    """P tile [k-in-block-c (part), q-in-block-a (free)]."""
                    p16 = P16[h]
                    if c == a:
                        return p16[:, 128 * c:128 * (c + 1)]
                    if c < a:
                        u = UIDX[(c, a)]
                        return p16[:, 512 + 128 * u:640 + 128 * u]
                    u = UIDX[(a, c)]
                    return TBs[h][:, 128 * u:128 * (u + 1)]

                def emit_pv(h, ablocks):
                    for a in ablocks:
                        base = 256 * a + 9 * h
                        for c in range(4):
                            nc.tensor.matmul(
                                psUA[:, base:base + 9], ptile(h, c, a),
                                vpc(h, c), start=False, stop=(c == 3),
                                skip_group_check=True)

                def emit_norm(h, ablocks):
                    a0, n = min(ablocks), len(ablocks)
                    zin = psUA[:].rearrange("p (a z) -> p a z", a=4)[
                        :, a0:a0 + n, 9 * h + 8]
                    nc.vector.reciprocal_approx_fast(
                        out=rz[:].rearrange("p (hh a) -> p hh a", hh=16)[
                            :, h, a0:a0 + n],
                        in_=zin)
                    o_in = psUA[:].rearrange("p (a z) -> p a z", a=4)[
                        :, a0:a0 + n, 9 * h:9 * h + 8]
                    rzb = rz[:].rearrange("p (hh a d) -> p hh a d", hh=16, a=4)[
                        :, h, a0:a0 + n, :].broadcast_to([128, n, 8])
                    xov = xo[:].rearrange("p (a e) -> p a e", a=4)[
                        :, a0:a0 + n, 8 * h:8 * h + 8]
                    nc.vector.tensor_tensor(xov, o_in, rzb, mybir.AluOpType.mult)

                def emit_norm3():
                    # merged normalization for heads 13..15 (the tail): one
                    # strided reciprocal + one broadcast multiply instead of
                    # three serialized per-head pairs
                    za = psUA[:].rearrange("p (a z) -> p a z", a=4)[
                        :, :, 117:144].rearrange("p a (hh d) -> p a hh d", hh=3)
                    rzv = rz[:, 52:64].rearrange("p (hh a) -> p a hh", hh=3)
                    nc.vector.reciprocal_approx_fast(
                        out=rzv, in_=za[:, :, :, 8])
                    rzb = rz[:, 52:64].rearrange(
                        "p (hh a d) -> p a hh d", hh=3, a=4).broadcast_to(
                        [128, 4, 3, 8])
                    xov = xo[:].rearrange("p (a e) -> p a e", a=4)[
                        :, :, 104:128].rearrange("p a (hh d) -> p a hh d", hh=3)
                    nc.vector.tensor_tensor(xov, za[:, :, :, 0:8], rzb,
                                            mybir.AluOpType.mult)

                def emit_out(ablocks, psTx, psOA, tag):
                    # transpose xo block -> project -> ship, per q-block
                    for a in ablocks:
                        nc.tensor.transpose(psTx[:, 128 * a:128 * (a + 1)],
                                            xo[:, 128 * a:128 * (a + 1)], idn)
                    xoT = cpool.tile([128, 512], FP16, tag=f"xoT{tag}")
                    lo, hi = 128 * min(ablocks), 128 * (max(ablocks) + 1)
                    nc.vector.tensor_copy(xoT[:, lo:hi], psTx[:, lo:hi])
                    for a in ablocks:
                        nc.tensor.matmul(psOA[:, 128 * a:128 * (a + 1)],
                                         xoT[:, 128 * a:128 * (a + 1)], w2,
                                         start=True, stop=False,
                                         skip_group_check=True)
                        nc.tensor.matmul(psOA[:, 128 * a:128 * (a + 1)],
                                         ones_r, bvec_r,
                                         start=False, stop=True,
                                         skip_group_check=True)
                    yo = cpool.tile([128, 512], FP32, tag=f"yo{tag}")
                    nc.vector.tensor_copy(yo[:, lo:hi], psOA[:, lo:hi])
                    nc.sync.dma_start(yout_d[:, lo:hi], yo[:, lo:hi])

                # ---- pipelined head loop
                LAG = 3
                for h in range(H + LAG + 1):
                    if h < H:
                        ps = emit_scores(h)
                        emit_exp(h, ps)
                        if h < H - 2:
                            emit_transpose(h)
                    hp = h - LAG
                    if 0 <= hp < H - 2:
                        emit_pv(hp, (0, 1, 2, 3))
                        if hp < H - 3:
                            emit_norm(hp, (0, 1, 2, 3))
                    elif hp == H - 2:
                        # heads 14+15: PE transposes share one PSUM tile whose
                        # ring slot frees at exp(14), so neither is gated by
                        # the other's ring position.
                        psT2[0] = psS.tile([128, 1536], FP16, tag="psS",
                                           name="psT2")
                        emit_transpose_pe_mm(hp, psT2[0][:, 0:768])
                        emit_transpose_pe_evac(hp, psT2[0][:, 0:768])
                        emit_pv(hp, (0, 1, 2, 3))
                    elif hp == H - 1:
                        # tail: PE-side transpose for the last head, then one
                        # combined normalize/project/ship chain. Ring slots
                        # (bufs=2): psT15 <- psS15 (exp15), psTx <- psT14
                        # (its evac), psOA <- psT15 (its evac) - no cycles.
                        psTx = psS.tile([128, 512], FP16, tag="psS",
                                        name="psTx")
                        psOA = psS.tile([128, 512], FP32, tag="psS",
                                        name="psOA")
                        emit_transpose_pe_mm(hp, psT2[0][:, 768:1536])
                        emit_transpose_pe_evac(hp, psT2[0][:, 768:1536])
                        emit_pv(hp, (0, 1, 2, 3))
                        emit_norm3()
                        emit_out((0, 1, 2, 3), psTx, psOA, "t")

    nc.compile()
    _CACHE[repeat] = nc
    return nc


def _xq(x: np.ndarray, theta: np.ndarray) -> np.ndarray:
    """Analytic quantum-head output: cosine prefix products, [B, S, H, NW]."""
    th = np.tile(theta.astype(np.float64), E // NW)
    C = np.cos(x.astype(np.float64) + th).reshape(B, S, H, NW)
    XQ = np.cumprod(C, axis=-1)
    XQ[..., 0] = np.prod(C[..., 1:], axis=-1)
    return XQ.astype(np.float32)


def _consts(W: np.ndarray, b: np.ndarray):
    cst = np.zeros((128, 640), dtype=np.float16)
    cst[:, 0:128] = W.astype(np.float16).T          # W2[8h+d, e'] = W[e', 8h+d]
    cst[:, 128:256] = np.eye(128, dtype=np.float16)  # identity for transposes
    cst[0, 256:384] = 1.0                            # ones row
    cst[0, 384:512] = b.astype(np.float16)           # bias
    return {"cst": cst}


def _prep_x(x: np.ndarray, theta: np.ndarray) -> list[dict]:
    """Per-core tensors: Q2T fp8 score operands + VP fp16 value slabs."""
    xq = _xq(x, theta)  # [B, S, H, NW]
    out = []
    for bb in range(B):
        xqb = xq[bb]  # [S, H, NW]
        # q2t[p, 1024h + 512i + s] = xq[s, h, 4i+p]
        q2 = xqb.astype(ml_dtypes.float8_e4m3)  # [S, H, NW]
        q2t = q2.transpose(1, 2, 0).reshape(H, 2, 4, S)  # [h, i, p, s]
        q2t = np.ascontiguousarray(q2t.transpose(2, 0, 1, 3)).reshape(4, H * 1024)
        # vp[p, 144c + 9h + d] = xq[128c+p, h, d] (d<8); 1.0 at d=8
        vp = np.ones((TB, 128, H, 9), dtype=np.float16)
        vp[:, :, :, 0:8] = xqb.reshape(TB, 128, H, NW).astype(np.float16)
        vp = np.ascontiguousarray(vp.transpose(1, 0, 2, 3)).reshape(128, TB * H * 9)
        out.append({"q2t": q2t, "vp": vp})
    return out


def _in_maps(x, theta, W, b):
    consts = _consts(W, b)
    per_core = _prep_x(x, theta)
    return [{**consts, **pc} for pc in per_core]


def kernel(x: np.ndarray, theta: np.ndarray, W: np.ndarray, b: np.ndarray) -> np.ndarray:
    x = np.asarray(x, dtype=np.float32)
    theta = np.asarray(theta, dtype=np.float32)
    W = np.asarray(W, dtype=np.float32)
    b = np.asarray(b, dtype=np.float32)

    nc = build(repeat=1)
    in_maps = _in_maps(x, theta, W, b)
    res = bass_utils.run_bass_kernel_spmd(nc, in_maps, core_ids=list(range(8)))

    y = np.empty((B, S, E), dtype=np.float32)
    for c in range(B):
        # yout[p, 128a + e'] -> y[128a + p, e']
        y[c] = res.results[c]["yout"].reshape(128, 4, 128).transpose(
            1, 0, 2).reshape(S, E)
    return y
